# revision 13
# baseline (speedup 1.0000x reference)
"""GQA attention with BitLinear projections, RMSNorm+RoPE, tanh softcap.

Sharding: 8 cores = batch(2) x kv-group(4). Each core handles one batch
element and one kv head (+ its 4 query heads), computes a partial o-proj
against its 256 columns of wo, and the host sums the 8 partials.
"""

import sys

if "/opt/trn_rl_repo" not in sys.path:
    sys.path.insert(0, "/opt/trn_rl_repo")

import numpy as np

import concourse.bass as bass
import concourse.mybir as mybir
import concourse.tile as tile
from concourse import bacc
from concourse.bass_utils import run_bass_kernel_spmd
from concourse.masks import make_identity

B, T, D, H, KVH, HD = 2, 2048, 1024, 16, 4, 64
HEADS_PER_CORE = H // KVH  # 4
DC = HEADS_PER_CORE * HD  # 256 q-proj dim per core
N_CORES = 8
SOFTCAP = 50.0
EPS = 1e-6
P = 128

F32 = mybir.dt.float32
F32R = mybir.dt.float32r

# matmul dtypes (knobs)
QK_DT = F32R   # qT/kT tiles
PV_DT = F32R   # p and v tiles

_CACHE = {}


def _build(t_len, mask_mode):
    """mask_mode: 'none' | 'causal' | 'general'."""
    nt = t_len // P          # 128-row t slices
    ntc = t_len // 512       # 512-col t tiles
    ntp = max(t_len // 1024, 1)  # t chunk pairs (1024)
    tc_per_tp = ntc // ntp
    ns = t_len // P          # s chunks
    KO = D // P              # 8 contraction chunks

    nc = bacc.Bacc(None, target_bir_lowering=False)

    xT_d = nc.dram_tensor("xT", [D, t_len], F32, kind="ExternalInput")
    wqT_d = nc.dram_tensor("wqT", [D, DC], F32, kind="ExternalInput")
    wkvT_d = nc.dram_tensor("wkvT", [D, 2 * HD], F32, kind="ExternalInput")
    woT_d = nc.dram_tensor("woT", [DC, D], F32, kind="ExternalInput")
    cosq_d = nc.dram_tensor("cosq", [t_len, HD], F32, kind="ExternalInput")
    sinq_d = nc.dram_tensor("sinq", [t_len, HD], F32, kind="ExternalInput")
    cosk_d = nc.dram_tensor("cosk", [t_len, HD], F32, kind="ExternalInput")
    sink_d = nc.dram_tensor("sink", [t_len, HD], F32, kind="ExternalInput")
    if mask_mode != "none":
        # mask already transposed to [s, t] and divided by SOFTCAP on host
        maskT_d = nc.dram_tensor("maskT", [t_len, t_len], F32, kind="ExternalInput")
    y_d = nc.dram_tensor("y", [t_len, D], F32, kind="ExternalOutput")

    AF = mybir.ActivationFunctionType

    with tile.TileContext(nc) as tc:
        with (
            tc.tile_pool(name="const", bufs=1) as constp,
            tc.tile_pool(name="big", bufs=1) as bigp,
            tc.tile_pool(name="work", bufs=2) as workp,
            tc.tile_pool(name="normp", bufs=2) as normp,
            tc.tile_pool(name="tbp", bufs=2) as tbp,
            tc.tile_pool(name="pbp", bufs=2) as pbp,
            tc.tile_pool(name="outp", bufs=1) as outp,
            tc.tile_pool(name="stage", bufs=2) as stagep,
            tc.tile_pool(name="psum_s", bufs=4, space="PSUM") as psum_s,
            tc.tile_pool(name="psum_qk", bufs=1, space="PSUM") as psum_qk,
        ):
            ident = constp.tile([P, P], F32)
            make_identity(nc, ident)

            # ---- persistent loads ----
            xT_sb = bigp.tile([P, KO, t_len], F32, tag="xT")
            nc.sync.dma_start(xT_sb[:], xT_d.rearrange("(o p) t -> p o t", p=P))
            wqT_sb = bigp.tile([P, KO, DC], F32, tag="wqT")
            nc.sync.dma_start(wqT_sb[:], wqT_d.rearrange("(o p) d -> p o d", p=P))
            wkvT_sb = bigp.tile([P, KO, 2 * HD], F32, tag="wkvT")
            nc.sync.dma_start(wkvT_sb[:], wkvT_d.rearrange("(o p) d -> p o d", p=P))
            woT_sb = bigp.tile([P, 2, D], F32, tag="woT")
            nc.sync.dma_start(woT_sb[:], woT_d.rearrange("(o p) e -> p o e", p=P))
            cs_sb = {}
            for name, dram in (("cq", cosq_d), ("sq", sinq_d),
                               ("ck", cosk_d), ("sk", sink_d)):
                cs_sb[name] = bigp.tile([P, nt, HD], F32, tag=name, name=name)
                nc.sync.dma_start(cs_sb[name][:],
                                  dram.rearrange("(o p) d -> p o d", p=P))

            qT_sb = bigp.tile([P, 2, t_len], QK_DT, tag="qT")
            kT_sb = bigp.tile([P, t_len], QK_DT, tag="kT")
            v_sb = bigp.tile([P, ns, HD + 2], PV_DT, tag="v")
            nc.vector.memset(v_sb[:].bitcast(F32), 1.0)

            MAGIC = 0x5F375A86

            def proj_slice(i):
                """QKV projection + rmsnorm + rope + transpose for t-slice i."""
                q_ps = psum_s.tile([P, DC], F32, tag="ps")
                kv_ps = psum_s.tile([P, 2 * HD], F32, tag="ps")
                for ko in range(KO):
                    nc.tensor.matmul(q_ps[:], xT_sb[:, ko, i * P:(i + 1) * P],
                                     wqT_sb[:, ko, :],
                                     start=(ko == 0), stop=(ko == KO - 1))
                for ko in range(KO):
                    nc.tensor.matmul(kv_ps[:], xT_sb[:, ko, i * P:(i + 1) * P],
                                     wkvT_sb[:, ko, :],
                                     start=(ko == 0), stop=(ko == KO - 1))

                # copy to SBUF (frees psum; DVE can't read 2 psum inputs)
                q_sb = workp.tile([P, DC], F32, tag="qsb")
                nc.vector.tensor_copy(q_sb[:], q_ps[:])
                kv_sb = workp.tile([P, 2 * HD], F32, tag="kvsb")
                nc.vector.tensor_copy(kv_sb[:], kv_ps[:])

                # mean-square: m = eps + sum(q^2)/HD  (per head)
                m_all = normp.tile([P, 5], F32, tag="m")
                scr = normp.tile([P, HEADS_PER_CORE, HD], F32, tag="scr")
                nc.vector.tensor_tensor(
                    scr[:], q_sb[:].rearrange("p (h d) -> p h d", d=HD),
                    q_sb[:].rearrange("p (h d) -> p h d", d=HD),
                    op=mybir.AluOpType.mult)
                nc.vector.tensor_reduce(m_all[:, 0:HEADS_PER_CORE], scr[:],
                                        axis=mybir.AxisListType.X,
                                        op=mybir.AluOpType.add)
                scrk = normp.tile([P, HD], F32, tag="scrk")
                nc.vector.tensor_tensor(scrk[:], kv_sb[:, 0:HD],
                                        kv_sb[:, 0:HD],
                                        op=mybir.AluOpType.mult)
                nc.vector.tensor_reduce(m_all[:, 4:5], scrk[:],
                                        axis=mybir.AxisListType.X,
                                        op=mybir.AluOpType.add)
                nc.vector.tensor_scalar(m_all[:], m_all[:], 1.0 / HD, EPS,
                                        op0=mybir.AluOpType.mult,
                                        op1=mybir.AluOpType.add)

                # rinv = rsqrt(m): bit-trick seed + 3 Newton iters (DVE only)
                y_t = normp.tile([P, 5], F32, tag="yt")
                y_int = y_t[:].bitcast(mybir.dt.int32)
                magic = normp.tile([P, 5], mybir.dt.int32, tag="magic")
                nc.vector.memset(magic[:], MAGIC)
                nc.vector.tensor_scalar(y_int, m_all[:].bitcast(mybir.dt.int32),
                                        1, None,
                                        op0=mybir.AluOpType.logical_shift_right)
                nc.vector.tensor_tensor(y_int, magic[:], y_int,
                                        op=mybir.AluOpType.subtract)
                t1 = normp.tile([P, 5], F32, tag="t1")
                for _ in range(3):
                    nc.vector.tensor_tensor(t1[:], y_t[:], y_t[:],
                                            op=mybir.AluOpType.mult)
                    nc.vector.tensor_tensor(t1[:], m_all[:], t1[:],
                                            op=mybir.AluOpType.mult)
                    nc.vector.tensor_scalar(t1[:], t1[:], -0.5, 1.5,
                                            op0=mybir.AluOpType.mult,
                                            op1=mybir.AluOpType.add)
                    nc.vector.tensor_tensor(y_t[:], y_t[:], t1[:],
                                            op=mybir.AluOpType.mult)

                # apply rinv (per-partition scalar per head)
                qn = workp.tile([P, HEADS_PER_CORE, HD], F32, tag="qn")
                for h in range(HEADS_PER_CORE):
                    nc.vector.tensor_scalar(qn[:, h, :],
                                            q_sb[:, h * HD:(h + 1) * HD],
                                            y_t[:, h:h + 1], None,
                                            op0=mybir.AluOpType.mult)
                kn = workp.tile([P, HD], F32, tag="kn")
                nc.vector.tensor_scalar(kn[:], kv_sb[:, 0:HD],
                                        y_t[:, 4:5], None,
                                        op0=mybir.AluOpType.mult)

                HH = HD // 2
                # rope for q (cos/sin broadcast across heads)
                rq = workp.tile([P, HEADS_PER_CORE, HD], F32, tag="rq")
                ta = workp.tile([P, HEADS_PER_CORE, HH], F32, tag="ta")
                c_lo = cs_sb["cq"][:, i:i + 1, 0:HH].to_broadcast(
                    (P, HEADS_PER_CORE, HH))
                s_lo = cs_sb["sq"][:, i:i + 1, 0:HH].to_broadcast(
                    (P, HEADS_PER_CORE, HH))
                c_hi = cs_sb["cq"][:, i:i + 1, HH:HD].to_broadcast(
                    (P, HEADS_PER_CORE, HH))
                s_hi = cs_sb["sq"][:, i:i + 1, HH:HD].to_broadcast(
                    (P, HEADS_PER_CORE, HH))
                nc.vector.tensor_tensor(rq[:, :, 0:HH], qn[:, :, 0:HH], c_lo,
                                        op=mybir.AluOpType.mult)
                nc.vector.tensor_tensor(ta[:], qn[:, :, HH:HD], s_lo,
                                        op=mybir.AluOpType.mult)
                nc.vector.tensor_tensor(rq[:, :, 0:HH], rq[:, :, 0:HH], ta[:],
                                        op=mybir.AluOpType.subtract)
                nc.vector.tensor_tensor(rq[:, :, HH:HD], qn[:, :, HH:HD], c_hi,
                                        op=mybir.AluOpType.mult)
                nc.vector.tensor_tensor(ta[:], qn[:, :, 0:HH], s_hi,
                                        op=mybir.AluOpType.mult)
                nc.vector.tensor_tensor(rq[:, :, HH:HD], rq[:, :, HH:HD], ta[:],
                                        op=mybir.AluOpType.add)
                # rope for k
                rk = workp.tile([P, HD], F32, tag="rk")
                tk = workp.tile([P, HH], F32, tag="tk")
                nc.vector.tensor_tensor(rk[:, 0:HH], kn[:, 0:HH],
                                        cs_sb["ck"][:, i, 0:HH],
                                        op=mybir.AluOpType.mult)
                nc.vector.tensor_tensor(tk[:], kn[:, HH:HD],
                                        cs_sb["sk"][:, i, 0:HH],
                                        op=mybir.AluOpType.mult)
                nc.vector.tensor_tensor(rk[:, 0:HH], rk[:, 0:HH], tk[:],
                                        op=mybir.AluOpType.subtract)
                nc.vector.tensor_tensor(rk[:, HH:HD], kn[:, HH:HD],
                                        cs_sb["ck"][:, i, HH:HD],
                                        op=mybir.AluOpType.mult)
                nc.vector.tensor_tensor(tk[:], kn[:, 0:HH],
                                        cs_sb["sk"][:, i, HH:HD],
                                        op=mybir.AluOpType.mult)
                nc.vector.tensor_tensor(rk[:, HH:HD], rk[:, HH:HD], tk[:],
                                        op=mybir.AluOpType.add)

                # transposes -> qT, kT (dup), v copy
                for mc in range(2):
                    t_ps = psum_s.tile([P, P], F32, tag="ps")
                    nc.tensor.transpose(t_ps[:], rq[:, 2 * mc:2 * mc + 2, :],
                                        ident[:])
                    nc.vector.tensor_copy(qT_sb[:, mc, i * P:(i + 1) * P],
                                          t_ps[:])
                tk_ps = psum_s.tile([HD, P], F32, tag="ps")
                nc.tensor.transpose(tk_ps[:], rk[:], ident[:])
                nc.vector.tensor_copy(kT_sb[0:HD, i * P:(i + 1) * P], tk_ps[:])
                nc.vector.tensor_copy(kT_sb[HD:P, i * P:(i + 1) * P], tk_ps[:])
                nc.vector.tensor_copy(v_sb[:, i, 0:HD], kv_sb[:, HD:2 * HD])

            def attn(hp, tp):
                """Attention for head pair hp over t chunk tp (1024 cols).

                Returns outT tile [128, tw]: rows 0-63 head 2hp, 64-127
                head 2hp+1 (matches o-proj lhsT layout).
                """
                t0 = tp * tc_per_tp * 512
                tw = tc_per_tp * 512
                ow = outp.tile([P, tw], F32, tag=f"ot_{hp}_{tp}",
                               name=f"ot_{hp}_{tp}")
                pv_ps = [psum_s.tile([P, 512], F32, tag="ps",
                                      name=f"pvps{_j}")
                         for _j in range(2 * tc_per_tp)]
                if mask_mode == "causal":
                    s_list = [s for s in range(ns) if s * P <= t0 + tw - 1]
                else:
                    s_list = list(range(ns))
                for si, s in enumerate(s_list):
                    qk_ps = psum_qk.tile([P, 2, tc_per_tp, 512], F32, tag="qk")
                    for j in range(2):
                        for tci in range(tc_per_tp):
                            nc.tensor.matmul(
                                qk_ps[:, j, tci, :],
                                kT_sb[HD * j:HD * (j + 1), s * P:(s + 1) * P],
                                qT_sb[HD * j:HD * (j + 1), hp,
                                      t0 + tci * 512:t0 + (tci + 1) * 512],
                                start=True, stop=True,
                                tile_position=(HD * j, 0))
                    tb = tbp.tile([P, 2, tc_per_tp, 512], F32, tag="tb")
                    nc.scalar.activation(tb[:], qk_ps[:], AF.Tanh,
                                         scale=1.0 / (8.0 * SOFTCAP))
                    if mask_mode != "none":
                        # add maskT/softcap for straddling tiles (causal) or all
                        if mask_mode == "general" or s * P + P > t0:
                            mt = stagep.tile([P, tc_per_tp, 512], F32, tag="mt")
                            nc.sync.dma_start(
                                mt[:], maskT_d[s * P:(s + 1) * P, t0:t0 + tw]
                                .rearrange("p (c f) -> p c f", f=512))
                            nc.vector.tensor_tensor(
                                tb[:], tb[:],
                                mt[:, None, :, :].to_broadcast(
                                    (P, 2, tc_per_tp, 512)),
                                op=mybir.AluOpType.add)
                    pb = pbp.tile([P, 2, tc_per_tp, 512], PV_DT, tag="pb")
                    nc.scalar.activation(pb[:], tb[:], AF.Exp, scale=SOFTCAP)
                    for j in range(2):
                        for tci in range(tc_per_tp):
                            nc.tensor.matmul(
                                pv_ps[j * tc_per_tp + tci][0:HD + 1, :],
                                v_sb[:, s, 0:HD + 1],
                                pb[:, j, tci, :],
                                start=(si == 0), stop=(si == len(s_list) - 1))
                # normalize -> outT (rows 64j..64j+63 for head j of pair)
                for j in range(2):
                    for tci in range(tc_per_tp):
                        rb = stagep.tile([HD, 512], F32, tag="rb", bufs=2)
                        nc.vector.reciprocal(rb[0:1, :],
                                             pv_ps[j * tc_per_tp + tci][HD:HD + 1, :])
                        nc.gpsimd.partition_broadcast(rb[:], rb[0:1, :],
                                                      channels=HD)
                        nc.vector.tensor_tensor(
                            ow[HD * j:HD * (j + 1),
                               tci * 512:(tci + 1) * 512],
                            pv_ps[j * tc_per_tp + tci][0:HD, :],
                            rb[:],
                            op=mybir.AluOpType.mult)
                return ow

            # ---- phase 1: kv/q proj for all slices ----
            for i in range(nt):
                proj_slice(i)

            # ---- phase 2+3 ----
            for tp in range(ntp):
                ow_by_hp = [attn(hp, tp) for hp in range(2)]
                for ii in range(tc_per_tp * 4):
                    gi = tp * tc_per_tp * 4 + ii
                    for nh in range(2):
                        op_ps = psum_s.tile([P, 512], F32, tag="ps")
                        for ko in range(2):
                            nc.tensor.matmul(
                                op_ps[:],
                                ow_by_hp[ko][:, ii * P:(ii + 1) * P],
                                woT_sb[:, ko, nh * 512:(nh + 1) * 512],
                                start=(ko == 0), stop=(ko == 1))
                        o_sb = stagep.tile([P, 512], F32, tag="osb", bufs=3)
                        nc.vector.tensor_copy(o_sb[:], op_ps[:])
                        nc.sync.dma_start(
                            y_d.rearrange("(o p) e -> p o e",
                                          p=P)[:, gi, nh * 512:(nh + 1) * 512],
                            o_sb[:])

    nc.finalize()
    return nc


def _get_nc(t_len, mask_mode):
    key = (t_len, mask_mode)
    if key not in _CACHE:
        _CACHE[key] = _build(t_len, mask_mode)
    return _CACHE[key]


def _host_prep(x, cos, sin, mask, wq, wk, wv, wo, q_norm_w, k_norm_w, t_len):
    f = np.float32
    wq, wk, wv, wo = (np.asarray(a, f) for a in (wq, wk, wv, wo))
    x = np.asarray(x, f)
    cos, sin = np.asarray(cos, f), np.asarray(sin, f)
    qw, kw = np.asarray(q_norm_w, f), np.asarray(k_norm_w, f)

    def eff(w):
        alpha = np.mean(np.abs(w), dtype=f)
        return (np.sign(w) * alpha).astype(f)

    wqe, wke, wve, woe = eff(wq), eff(wk), eff(wv), eff(wo)

    qw_sw = np.concatenate([qw[HD // 2:], qw[:HD // 2]])
    kw_sw = np.concatenate([kw[HD // 2:], kw[:HD // 2]])
    cosq = np.ascontiguousarray(cos * qw[None, :])
    sinq = np.ascontiguousarray(sin * qw_sw[None, :])
    cosk = np.ascontiguousarray(cos * kw[None, :])
    sink = np.ascontiguousarray(sin * kw_sw[None, :])

    m2 = np.asarray(mask, f).reshape(t_len, t_len)
    if not np.any(m2):
        mask_mode = "none"
        maskT = None
    else:
        causal = np.array_equal(
            m2, np.where(np.tril(np.ones((t_len, t_len), bool)), f(0), f(-1e9)))
        mask_mode = "causal" if causal else "general"
        maskT = np.ascontiguousarray(m2.T) / f(SOFTCAP)

    in_maps = []
    for c in range(N_CORES):
        b, g = divmod(c, KVH)
        im = {
            "xT": np.ascontiguousarray(x[b].T),
            "wqT": np.ascontiguousarray(wqe[g * DC:(g + 1) * DC, :].T),
            "wkvT": np.ascontiguousarray(
                np.concatenate([wke[g * HD:(g + 1) * HD, :],
                                wve[g * HD:(g + 1) * HD, :]], axis=0).T),
            "woT": np.ascontiguousarray(woe.T[g * DC:(g + 1) * DC, :]),
            "cosq": cosq, "sinq": sinq, "cosk": cosk, "sink": sink,
        }
        if maskT is not None:
            im["maskT"] = maskT
        in_maps.append(im)
    return in_maps, mask_mode


def kernel(x, cos, sin, mask, wq, wk, wv, wo, q_norm_w, k_norm_w,
           _trace=False, _t_len=T):
    in_maps, mask_mode = _host_prep(x, cos, sin, mask, wq, wk, wv, wo,
                                    q_norm_w, k_norm_w, _t_len)
    nc = _get_nc(_t_len, mask_mode)
    res = run_bass_kernel_spmd(nc, in_maps, core_ids=list(range(N_CORES)),
                               trace=_trace)
    out = np.zeros((B, _t_len, D), np.float32)
    for c in range(N_CORES):
        b = c // KVH
        out[b] += res.results[c]["y"]
    if _trace:
        kernel._last = res
    return out
